# revision 42
# baseline (speedup 1.0000x reference)
"""Diagonal-masked multi-head self-attention on 8 TRN2 NeuronCores.

Sharding: core c handles batch b = c // 2 and heads h0 = (c % 2) * 8 .. +8
(data parallel on B=4, tensor parallel over the 16 heads).  Each core
computes a partial output [S, D]; the host sums the two half-head partials
per batch and adds the output bias.

v6 schedule, built around three hardware facts:
  * ScalarE exp runs 1 elem/cycle/lane: the 256 [128,1024] exp tiles are
    a ~285us floor, so every other engine hides under the exp stream.
  * DK=64: the two heads of a pair are row-tiled onto the top/bottom
    64-row halves of the PE array (tile_position via base partitions) so
    a score-tile pair costs ~one matmul slot.
  * The Sync engine dispatches DMAs serially (~0.6us each), so DMAs are
    batched one-instruction-per-chunk via (k p) c access patterns, and
    xk stays resident in SBUF for all pairs' K projections.

Emission: a minimal head (wk+xk -> K(p0), wq+xq0 -> Q(p0,c0)) starts the
exp stream at ~15us.  The full V projection and all remaining K/Q
projections run as fillers inside the 256-iteration exp stream, at most
one K and one Q projection chain per stage so the shared PSUM rings keep
the baseline's proven cadence.  Each chunk's normalization is emitted at
the top of the NEXT chunk so a lagging PV accumulation never
head-of-line-blocks the DVE queue.  In the last stage the output
projection runs chunk-by-chunk as fillers.
"""

import numpy as np
import ml_dtypes

B, S, D, H = 4, 2048, 1024, 16
DK = D // H
N_CORES = 8
HEADS_PER_CORE = H // 2


def build_attention_core(S=2048, DIN=1024, NH=8, DOUT=1024, aug_bias=False,
                         debug_taps=False):
    import concourse.bacc as bacc
    import concourse.bass as bass
    import concourse.mybir as mybir
    import concourse.tile as tile

    fp32 = mybir.dt.float32
    bf16 = mybir.dt.bfloat16

    NP = NH // 2              # head pairs
    DC = NH * DK              # concat head dim on this core
    VW = 72                   # per-head V slot: [V(64) ones(1) pad(7)]
    VU = DK + 1               # used columns of a V slot
    NT = S // 128             # t tiles (key/value positions)
    NQ = S // 512             # q chunks of 512
    KA = DIN + 1 if aug_bias else DIN
    NKF = DIN // 128          # full 128-row contraction tiles
    NK = NKF + (1 if aug_bias else 0)
    QT = S // 128             # output q tiles

    assert S % 512 == 0 and DIN % 128 == 0 and DOUT == 1024

    nc = bacc.Bacc(None, target_bir_lowering=False, debug=False)

    xq = nc.dram_tensor("xq", [KA, S], bf16, kind="ExternalInput")
    xk = nc.dram_tensor("xk", [KA, S], bf16, kind="ExternalInput")
    xv = nc.dram_tensor("xv", [KA, S], bf16, kind="ExternalInput")
    wq = nc.dram_tensor("wq", [KA, DC], bf16, kind="ExternalInput")
    wk = nc.dram_tensor("wk", [KA, DC], bf16, kind="ExternalInput")
    wv = nc.dram_tensor("wv", [KA, DC], bf16, kind="ExternalInput")
    wo = nc.dram_tensor("wo", [DC, DOUT], bf16, kind="ExternalInput")
    dmk = nc.dram_tensor("dmk", [128, 4 * 1024], bf16, kind="ExternalInput")
    outp = nc.dram_tensor("outp", [S, DOUT], fp32, kind="ExternalOutput")

    def ksz(k):  # rows in contraction tile k
        return min(128, KA - k * 128)

    scale = float(1.0 / np.sqrt(DK))

    with tile.TileContext(nc) as tc:
        with (
            tc.tile_pool(name="persist", bufs=1) as persist,
            tc.tile_pool(name="vxp", bufs=2) as vxp,
            tc.tile_pool(name="qxp", bufs=2) as qxp,
            tc.tile_pool(name="win", bufs=1) as win,
            tc.tile_pool(name="epool", bufs=5) as epool,
            tc.tile_pool(name="npool", bufs=2) as npool,
            tc.tile_pool(name="opool", bufs=2) as opool,
            tc.tile_pool(name="scps", bufs=2, space="PSUM") as scps,
            tc.tile_pool(name="otaps", bufs=2, space="PSUM") as otaps,
            tc.tile_pool(name="otbps", bufs=2, space="PSUM") as otbps,
        ):
            # ---- persistent SBUF tensors -------------------------------
            qht = persist.tile([128, NP * S], bf16, tag="qht")        # pair-major
            kht = persist.tile([128, NP * S], bf16, tag="kht")        # pair-major
            vh = persist.tile([128, NH * NT * VW + 56], bf16, tag="vh")  # +pad for 128-col lhsT
            ot = persist.tile([128, NP * S], bf16, tag="ot")
            dmask = persist.tile([128, 4 * 1024], bf16, tag="dmask")
            wo_sb = persist.tile([128, NP * DOUT], bf16, tag="wo")
            xkr = persist.tile([128, NKF * S], bf16, tag="xkr")       # resident xk
            if aug_bias:
                xka = persist.tile([1, S], bf16, tag="xka")

            vh4 = vh[:, 0: NH * NT * VW].rearrange("p (h t c) -> p h t c", t=NT, c=VW)
            xkr3 = xkr.rearrange("p (k c) -> p k c", c=S)
            xk3 = xk[0: NKF * 128, :].rearrange("(k p) c -> p k c", p=128)

            def xkr_ap(k, rows, c0, c1):
                """Contraction tile k of resident xk, absolute cols c0:c1."""
                if k < NKF:
                    return xkr[0:rows, k * S + c0: k * S + c1]
                return xka[0:rows, c0:c1]

            # ---- batched DMA helpers (one Sync instr per ~1MB) ---------
            wtl = {}

            def dma_w(which, wdram):
                wall = win.tile([128, NKF * DC], bf16, tag=f"w{which}")
                nc.sync.dma_start(
                    wall.rearrange("p (k c) -> p k c", c=DC),
                    wdram[0: NKF * 128, :].rearrange("(k p) c -> p k c", p=128),
                )
                wtl[which] = wall
                if aug_bias:
                    wx = win.tile([1, DC], bf16, tag=f"w{which}x")
                    nc.sync.dma_start(wx[:], wdram[NKF * 128: KA, :])
                    wtl[which + "x"] = wx

            def wap(which, k, rows, c0, c1):
                """Contraction tile k of weight `which`, cols c0:c1 of DC."""
                if k < NKF:
                    return wtl[which][0:rows, k * DC + c0: k * DC + c1]
                return wtl[which + "x"][0:1, c0:c1]

            def dma_xc(xdram, n, pool, tag):
                """One x chunk (all contraction tiles) as a single DMA."""
                big = pool.tile([128, NKF * 512], bf16, tag=tag)
                nc.sync.dma_start(
                    big.rearrange("p (k c) -> p k c", c=512),
                    xdram[0: NKF * 128, n * 512:(n + 1) * 512].rearrange(
                        "(k p) c -> p k c", p=128
                    ),
                )
                aug_t = None
                if aug_bias:
                    aug_t = pool.tile([1, 512], bf16, tag=tag + "a")
                    nc.sync.dma_start(
                        aug_t[:], xdram[NKF * 128: KA, n * 512:(n + 1) * 512]
                    )

                def xap(k, rows, c0=0, c1=512):
                    if k < NKF:
                        return big[0:rows, k * 512 + c0: k * 512 + c1]
                    return aug_t[0:rows, c0:c1]

                return xap

            # ---------------- projection emitters -----------------------
            def proj_kq_fillers(which, p, n, xap):
                """Closures (one matmul each) for a K/Q projection chain."""
                pool = otbps if which == "q" else otaps
                tag = "otb" if which == "q" else "ota"
                box = {}

                def mk(k):
                    def emit():
                        if k == 0:
                            box["ps"] = pool.tile([128, 512], fp32, tag=tag, name="pjps")
                        ps = box["ps"]
                        nc.tensor.matmul(
                            ps[:],
                            wap(which, k, ksz(k), p * 128, (p + 1) * 128),
                            xap(k, ksz(k)),
                            start=(k == 0),
                            stop=(k == NK - 1),
                        )
                        if k == NK - 1:
                            dst = qht if which == "q" else kht
                            nc.vector.tensor_copy(
                                dst[:, p * S + n * 512: p * S + (n + 1) * 512],
                                ps[:],
                            )

                    return emit

                return [mk(k) for k in range(NK)]

            def proj_kq(which, p, n, xap):
                for f in proj_kq_fillers(which, p, n, xap):
                    f()

            def proj_kq_xkr(which, p, n):
                """K projection chain reading the resident xk."""
                c0, c1 = n * 512, (n + 1) * 512
                return proj_kq_fillers(
                    which, p, n, lambda k, rows, a=0, b=512: xkr_ap(k, rows, c0 + a, c0 + b)
                )

            def proj_v_fillers(n, xap, pre=None):
                """Closures (one t-tile each) for chunk n's V projection."""

                def mk(tt):
                    def emit():
                        if pre is not None and tt == 0:
                            pre()
                        t = n * 4 + tt
                        ps = scps.tile([128, 1024], fp32, tag="sc", name="vps")
                        for k in range(NK):
                            nc.tensor.matmul(
                                ps[:, 0:512],
                                xap(k, ksz(k), tt * 128, (tt + 1) * 128),
                                wap("v", k, ksz(k), 0, DC),
                                start=(k == 0),
                                stop=(k == NK - 1),
                            )
                        nc.vector.tensor_copy(
                            vh4[:, :, t, 0:DK],
                            ps[:, 0:512].rearrange("p (h c) -> p h c", c=DK),
                        )

                    return emit

                return [mk(tt) for tt in range(4)]

            def sc_mm(p, n):
                """Emit the score matmul pair for (pair p, chunk n, tile t).

                DK=64: the two heads of a pair are row-tiled onto halves of
                the PE array (tile_position (0,0)/(64,0), inferred from the
                base partitions) and run concurrently."""
                qof = p * S + n * 512
                kof = p * S

                def emit(t):
                    sc = scps.tile([128, 1024], fp32, tag="sc")
                    nc.tensor.matmul(
                        sc[:, 0:512],
                        kht[0:64, kof + t * 128: kof + (t + 1) * 128],
                        qht[0:64, qof: qof + 512],
                        start=True, stop=True,
                    )
                    nc.tensor.matmul(
                        sc[:, 512:1024],
                        kht[64:128, kof + t * 128: kof + (t + 1) * 128],
                        qht[64:128, qof: qof + 512],
                        start=True, stop=True,
                    )
                    return sc

                return emit

            pend = {}

            def attn_chunk(p, n, fillers=(), nxt=None, prev_norm=None):
                """Attention for pair p, q-chunk n (512 q positions).

                fillers: closures, each ~1 PE matmul, interleaved into the
                t-loop to fill the exp-wait slack.
                nxt: the following chunk; its first score pair is emitted
                before this chunk's last PV so ScalarE never idles across
                the boundary.
                prev_norm: previous chunk's normalization closure, emitted
                here so it can never head-of-line-block the DVE queue.
                Returns this chunk's normalization closure."""
                qof = p * S + n * 512
                ota = otaps.tile([128, 512], fp32, tag="ota")
                otb = otbps.tile([128, 512], fp32, tag="otb")
                mine = sc_mm(p, n)
                sc_cur = pend.pop("sc", None)
                if sc_cur is None:
                    sc_cur = mine(0)
                if prev_norm is not None:
                    prev_norm()
                fq = list(fillers)
                nf = len(fq)
                emitted = 0
                for t in range(NT):
                    e = epool.tile([128, 1024], bf16, tag="e")
                    nc.scalar.activation(
                        e[:], sc_cur[:], mybir.ActivationFunctionType.Exp,
                        scale=scale,
                    )
                    off = t * 128 - n * 512
                    if 0 <= off < 512:
                        d = off // 128
                        nc.vector.tensor_mul(
                            e[:], e[:], dmask[:, d * 1024:(d + 1) * 1024]
                        )
                    if t < NT - 1:
                        sc_cur = mine(t + 1)
                    while emitted < ((t + 1) * nf) // NT:
                        fq[emitted]()
                        emitted += 1
                    va = ((2 * p) * NT + t) * VW
                    vb = ((2 * p + 1) * NT + t) * VW
                    nc.tensor.matmul(
                        ota[:], vh[:, va: va + 128], e[:, 0:512],
                        start=(t == 0), stop=(t == NT - 1),
                    )
                    nc.tensor.matmul(
                        otb[:], vh[:, vb: vb + 128], e[:, 512:1024],
                        start=(t == 0), stop=(t == NT - 1),
                    )
                while emitted < nf:
                    fq[emitted]()
                    emitted += 1
                # the cross-chunk score lookahead MUST be emitted after the
                # filler flush: emission order is semantic, and this stage's
                # fillers may include the projection copy the next chunk's
                # first score matmul reads.
                if nxt is not None:
                    pend["sc"] = sc_mm(*nxt)(0)

                def norm():
                    # normalize (denominators on PSUM row 64)
                    rd = npool.tile([128, 1024], fp32, tag="rd")
                    nc.vector.reciprocal_approx_fast(rd[:, 0:512], ota[:])
                    nc.vector.reciprocal_approx_fast(rd[:, 512:1024], otb[:])
                    nc.sync.dma_start(rd[0:1, 0:512], rd[64:65, 0:512])
                    nc.sync.dma_start(rd[0:1, 512:1024], rd[64:65, 512:1024])
                    bca = npool.tile([64, 512], fp32, tag="bca")
                    bcb = npool.tile([64, 512], fp32, tag="bcb")
                    nc.gpsimd.partition_broadcast(bca[:], rd[0:1, 0:512], channels=64)
                    nc.gpsimd.partition_broadcast(bcb[:], rd[0:1, 512:1024], channels=64)
                    nc.vector.tensor_mul(ot[0:64, qof: qof + 512], ota[0:64, :], bca[:])
                    tmpb = npool.tile([64, 512], bf16, tag="tmpb")
                    nc.vector.tensor_mul(tmpb[:], otb[0:64, :], bcb[:])
                    nc.sync.dma_start(ot[64:128, qof: qof + 512], tmpb[:])

                return norm

            def out_qt_closure(qt):
                """One output-projection q-subtile (8 matmuls + copy + DMA)."""

                def emit():
                    ps = scps.tile([128, 1024], fp32, tag="sc", name="cps")
                    for nd in range(2):
                        for p in range(NP):
                            nc.tensor.matmul(
                                ps[:, nd * 512:(nd + 1) * 512],
                                ot[:, p * S + qt * 128: p * S + (qt + 1) * 128],
                                wo_sb[:, p * DOUT + nd * 512: p * DOUT + nd * 512 + 512],
                                start=(p == 0), stop=(p == NP - 1),
                            )
                    osb = opool.tile([128, 1024], fp32, tag="osb", name="osb")
                    nc.vector.tensor_copy(osb[:], ps[:])
                    nc.sync.dma_start(outp[qt * 128:(qt + 1) * 128, :], osb[:])

                return emit

            # ---------------- emission ----------------------------------
            # head: minimal critical path to the first exp —
            # wk+xk(c0) -> K(p0,c0..); wq+xq0 -> Q(p0,c0); dmask; scores.
            dma_w("k", wk)
            nc.sync.dma_start(xkr3[:, :, 0:512], xk3[:, :, 0:512])
            dma_w("q", wq)
            qx0 = dma_xc(xq, 0, qxp, "qx")
            # xk chunk 1 right behind the first-score deps: the K(p0,c1)
            # chain sits between the first and second score pairs in the
            # PE FIFO, so its DMA gates exp(1).
            nc.sync.dma_start(xkr3[:, :, 512:1024], xk3[:, :, 512:1024])
            dma_w("v", wv)
            vxa = {0: dma_xc(xv, 0, vxp, "vx")}
            for n in range(2, NQ):
                nc.sync.dma_start(
                    xkr3[:, :, n * 512:(n + 1) * 512],
                    xk3[:, :, n * 512:(n + 1) * 512],
                )
            vxa[1] = dma_xc(xv, 1, vxp, "vx")
            nc.sync.dma_start(dmask[:], dmk[:])
            if aug_bias:
                nc.sync.dma_start(xka[:], xk[NKF * 128: KA, :])
            # only K(p0,c0) + Q(p0,c0) gate the first scores: emit the
            # first score pair into `pend` BEFORE the DMA-gated K chunks
            # 1-3, so the exp stream starts as soon as c0 is projected.
            for f in proj_kq_xkr("k", 0, 0):
                f()
            proj_kq("q", 0, 0, qx0)
            pend["sc"] = sc_mm(0, 0)(0)
            for n in range(1, NQ):
                for f in proj_kq_xkr("k", 0, n):
                    f()
            nc.vector.memset(vh4[:, :, :, 64:65], 1.0)  # ones columns only
            nc.sync.dma_start(
                wo_sb.rearrange("p (k c) -> p k c", c=DOUT),
                wo[:, :].rearrange("(k p) c -> p k c", p=128),
            )

            # V projection: all 16 t-tiles as stage-(0,0) fillers; PV
            # consumes tile t at iteration t.  Chunks 2,3 DMA'd from
            # inside the filler stream once earlier chunks are consumed.
            vfill = []
            for n in range(NQ):
                pre = None
                if 0 < n < NQ - 1:
                    def pre(_m=n + 1):
                        vxa[_m] = dma_xc(xv, _m, vxp, "vx")
                if n < 2:
                    vfill += proj_v_fillers(n, vxa[n], pre=pre)
                else:
                    vfill += proj_v_fillers(
                        n,
                        lambda k, rows, a=0, b=512, _n=n: vxa[_n](k, rows, a, b),
                        pre=pre,
                    )


            # per-stage K and Q projection chains (<=1 of each per stage,
            # matching the 2-buf PSUM ring cadence), K from resident xk.
            k_sched = {0: (1, 0), 1: (1, 1), 2: (1, 2), 3: (1, 3),
                       4: (2, 0), 5: (2, 1), 6: (2, 2), 7: (2, 3),
                       8: (3, 0), 9: (3, 1), 10: (3, 2), 11: (3, 3)}
            q_sched = {0: (0, 1), 1: (0, 2), 2: (0, 3), 3: (1, 0),
                       4: (1, 1), 5: (1, 2), 6: (1, 3), 7: (2, 0),
                       8: (2, 1), 9: (2, 2), 10: (2, 3), 11: (3, 0),
                       12: (3, 1), 13: (3, 2), 14: (3, 3)}

            order = [(p, n) for p in range(NP) for n in range(NQ)]
            # prefetch the xq chunk for stage idx's Q chain one stage early
            qx_pref = {0: dma_xc(xq, q_sched[0][1], qxp, "qx")}
            prev_norm = None
            for idx, (p, n) in enumerate(order):
                if idx + 1 in q_sched:
                    qx_pref[idx + 1] = dma_xc(xq, q_sched[idx + 1][1], qxp, "qx")
                fillers = []
                if idx == 0:
                    fillers += vfill
                if idx in k_sched:
                    kp, kn = k_sched[idx]
                    fillers += proj_kq_xkr("k", kp, kn)
                if idx in q_sched:
                    qp, qn = q_sched[idx]
                    fillers += proj_kq_fillers("q", qp, qn, qx_pref.pop(idx))
                if p == NP - 1 and n > 0:
                    fillers += [out_qt_closure(qt)
                                for qt in range((n - 1) * 4, n * 4)]
                nxt = order[idx + 1] if idx + 1 < len(order) else None
                prev_norm = attn_chunk(p, n, fillers, nxt, prev_norm)
            # final chunk's output projection: pairs 0-2 only need ot from
            # earlier stages, so their 6 matmuls per q-subtile accumulate
            # while norm(3,3) still runs; only pair 3's two matmuls (and
            # the copy/DMA) wait for the last normalization.
            last_qts = list(range((NQ - 1) * 4, NQ * 4))
            partial = {}
            for qt in last_qts[0:2]:
                ps = scps.tile([128, 1024], fp32, tag="sc", name="cps")
                for nd in range(2):
                    for p in range(NP - 1):
                        nc.tensor.matmul(
                            ps[:, nd * 512:(nd + 1) * 512],
                            ot[:, p * S + qt * 128: p * S + (qt + 1) * 128],
                            wo_sb[:, p * DOUT + nd * 512: p * DOUT + nd * 512 + 512],
                            start=(p == 0), stop=False,
                        )
                partial[qt] = ps
            prev_norm()
            for qt in last_qts:
                if qt in partial:
                    ps = partial[qt]
                    p = NP - 1
                    for nd in range(2):
                        nc.tensor.matmul(
                            ps[:, nd * 512:(nd + 1) * 512],
                            ot[:, p * S + qt * 128: p * S + (qt + 1) * 128],
                            wo_sb[:, p * DOUT + nd * 512: p * DOUT + nd * 512 + 512],
                            start=False, stop=True,
                        )
                    osb = opool.tile([128, 1024], fp32, tag="osb", name="osb")
                    nc.vector.tensor_copy(osb[:], ps[:])
                    nc.sync.dma_start(outp[qt * 128:(qt + 1) * 128, :], osb[:])
                else:
                    out_qt_closure(qt)()

            if debug_taps:
                dbg = {
                    "dbg_kht": kht, "dbg_qht": qht, "dbg_ot": ot,
                    "dbg_vh": vh, "dbg_xkr": xkr,
                }
                for name, t_ in dbg.items():
                    dt_ = nc.dram_tensor(
                        name, list(t_.shape), bf16, kind="ExternalOutput"
                    )
                    nc.sync.dma_start(dt_[:, :], t_[:, :])

    nc.compile()
    return nc


def _bf16(a):
    return np.ascontiguousarray(a).astype(ml_dtypes.bfloat16)


def _build_dmask():
    m = np.ones((128, 4 * 1024), np.float32)
    for d in range(4):
        for i in range(128):
            m[i, d * 1024 + d * 128 + i] = 0.0
            m[i, d * 1024 + 512 + d * 128 + i] = 0.0
    return _bf16(m)


def _prep_core_inputs(q, k, v, Wq, bq, Wk, bk, Wv, bv, Wo, aug_bias):
    """Per-core host-side slicing/transposition. Returns list of 8 dicts."""
    dmk = _build_dmask()
    maps = []
    for c in range(N_CORES):
        b = c // 2
        h0 = (c % 2) * HEADS_PER_CORE
        r0, r1 = h0 * DK, (h0 + HEADS_PER_CORE) * DK
        m = {}
        for name, x in (("xq", q[b]), ("xk", k[b]), ("xv", v[b])):
            xt = x.T  # [D, S]
            if aug_bias:
                xt = np.concatenate([xt, np.ones((1, S), np.float32)], axis=0)
            m[name] = _bf16(xt)
        for name, W, bias in (("wq", Wq, bq), ("wk", Wk, bk), ("wv", Wv, bv)):
            wtm = W[r0:r1, :].T  # [D, DC]
            if aug_bias:
                wtm = np.concatenate([wtm, bias[None, r0:r1]], axis=0)
            m[name] = _bf16(wtm)
        m["wo"] = _bf16(Wo[:, r0:r1].T)  # [DC, D]
        m["dmk"] = dmk
        maps.append(m)
    return maps


_PROGRAM_CACHE = {}


def _get_program(aug_bias):
    if aug_bias not in _PROGRAM_CACHE:
        _PROGRAM_CACHE[aug_bias] = build_attention_core(
            S=S, DIN=D, NH=HEADS_PER_CORE, DOUT=D, aug_bias=aug_bias
        )
    return _PROGRAM_CACHE[aug_bias]


def _reference_fallback(q, k, v, Wq, bq, Wk, bk, Wv, bv, Wo, bo, mask):
    """Pure-numpy fallback for unexpected mask patterns."""
    out = np.empty((B, S, D), np.float32)
    msk = np.broadcast_to(mask.reshape(mask.shape[-2], mask.shape[-1]), (S, S))
    for b in range(B):
        qh = (q[b] @ Wq.T + bq).reshape(S, H, DK).transpose(1, 0, 2)
        kh = (k[b] @ Wk.T + bk).reshape(S, H, DK).transpose(1, 0, 2)
        vh = (v[b] @ Wv.T + bv).reshape(S, H, DK).transpose(1, 0, 2)
        acc = np.empty((H, S, DK), np.float32)
        for h in range(H):
            s = (qh[h] @ kh[h].T) / np.float32(np.sqrt(DK))
            s = np.where(msk == 0, np.finfo(np.float32).min, s)
            s = s - s.max(axis=-1, keepdims=True)
            e = np.exp(s)
            p = e / e.sum(axis=-1, keepdims=True)
            acc[h] = p @ vh[h]
        o = acc.transpose(1, 0, 2).reshape(S, D)
        out[b] = o @ Wo.T + bo
    return out


def kernel(q, k, v, Wq, bq, Wk, bk, Wv, bv, Wo, bo, mask, _trace=False):
    from concourse.bass_utils import run_bass_kernel_spmd

    q = np.asarray(q, np.float32)
    k = np.asarray(k, np.float32)
    v = np.asarray(v, np.float32)
    Wq, bq = np.asarray(Wq, np.float32), np.asarray(bq, np.float32)
    Wk, bk = np.asarray(Wk, np.float32), np.asarray(bk, np.float32)
    Wv, bv = np.asarray(Wv, np.float32), np.asarray(bv, np.float32)
    Wo, bo = np.asarray(Wo, np.float32), np.asarray(bo, np.float32)
    mask = np.asarray(mask)

    expected_mask = 1 - np.eye(S, dtype=np.int32)
    if not np.array_equal(mask.reshape(-1, S, S)[0].astype(np.int32), expected_mask):
        return _reference_fallback(q, k, v, Wq, bq, Wk, bk, Wv, bv, Wo, bo, mask)

    aug_bias = bool(np.any(bq) or np.any(bk) or np.any(bv))
    nc = _get_program(aug_bias)
    in_maps = _prep_core_inputs(q, k, v, Wq, bq, Wk, bk, Wv, bv, Wo, aug_bias)
    res = run_bass_kernel_spmd(
        nc, in_maps, core_ids=list(range(N_CORES)), trace=_trace
    )
    out = np.empty((B, S, D), np.float32)
    for b in range(B):
        out[b] = res.results[2 * b]["outp"] + res.results[2 * b + 1]["outp"] + bo
    if _trace:
        kernel.last_results = res
    return out
